# revision 11
# baseline (speedup 1.0000x reference)
"""Trainium2 Bass kernel for nn_EnhancedMultiHeadDINAttention.

Math (algebraically collapsed from the reference):
  q = cand @ Wq + bq                          [B, D]
  scores[b,s] = x[b,s,:] . r_b + c_b          r = (Wk @ q)*SCALE, c = (q.bk)*SCALE
  scores -> mask(-1e9) -> * decay -> exp -> w_e (unnormalized; the softmax
  denominator folds into a per-batch scale at the end)
  xw[b,:]  = sum_s w[b,s] x[b,s,:]
  xs[b,:]  = mean_s x[b,s,:]
  attended = xw @ Wv + bv + cand
  kbar     = xs @ Wk + bk
  inter_mean = concat([q, kbar, q-kbar, q*kbar]) @ Wi + bi
  out = concat([attended, inter_mean], -1)    [B, 2D]

Sharding: pure data parallel over batch, 8 cores x 64 batches.

Per-core layout: x streams HBM->SBUF once as bf16 (SWDGE cast in the DMA)
into [128, 100, 256] tiles where partition p = 64h + b holds batch b,
s-half h -- each partition reads one contiguous f32 run (max DMA
efficiency, no staging, no transpose of x anywhere).

scores: DVE broadcast-mult (x * r~, r~ replicated per partition pair) +
innermost reduce, both at the 2x bf16/fp16 rate.

xw/xs: PE matmuls with x d-slices as the *stationary* and a per-s masked
moving operand WD[:, s, 0:64] = pairmask * w_e[:, s] (diag pairs),
WD[:, s, 64:128] = pairmask. psum accumulates [d', (xw~T | xsT)] over all
s -- transposed exactly as the final Wv/Wk matmuls want, and the softmax
denominator is applied per-partition when extracting attended.
"""

import numpy as np

import concourse.bass as bass
import concourse.bacc as bacc
import concourse.tile as tile
from concourse import mybir
from concourse.masks import make_identity

F32 = mybir.dt.float32
BF16 = mybir.dt.bfloat16
FP16 = mybir.dt.float16
I32 = mybir.dt.int32

N_CORES = 8
B, S, D = 512, 200, 256
BS = B // N_CORES          # 64 batches per core
HS = S // 2                # 100 s-values per half
NCH = 5                    # x-load s-chunks
SC = HS // NCH             # 20 s-values per chunk (x2 halves via partitions)
SCALE = 1.0 / np.sqrt(D // 4)   # 1/8
TIME_DECAY = 0.01
NEG_INF = -1e9

AF = mybir.ActivationFunctionType
ALU = mybir.AluOpType
AX = mybir.AxisListType


def _ap(ap, off, dims):
    return bass.AP(tensor=ap.tensor, offset=ap.offset + off, ap=dims)


def build_program():
    nc = bacc.Bacc("TRN2", target_bir_lowering=False, debug=False)

    past = nc.dram_tensor("past_interactions", [BS, S, D], F32, kind="ExternalInput").ap()
    cand = nc.dram_tensor("candidate_embedding", [BS, D], F32, kind="ExternalInput").ap()
    maskt = nc.dram_tensor("past_mask", [BS, S], I32, kind="ExternalInput").ap()
    tstamp = nc.dram_tensor("past_timestamps", [BS, S], F32, kind="ExternalInput").ap()
    Wq = nc.dram_tensor("Wq", [D, D], F32, kind="ExternalInput").ap()
    bq = nc.dram_tensor("bq", [D], F32, kind="ExternalInput").ap()
    Wk = nc.dram_tensor("Wk", [D, D], F32, kind="ExternalInput").ap()
    bk = nc.dram_tensor("bk", [D], F32, kind="ExternalInput").ap()
    Wv = nc.dram_tensor("Wv", [D, D], F32, kind="ExternalInput").ap()
    bv = nc.dram_tensor("bv", [D], F32, kind="ExternalInput").ap()
    Wi = nc.dram_tensor("Wi", [4 * D, D], F32, kind="ExternalInput").ap()
    bi = nc.dram_tensor("bi", [D], F32, kind="ExternalInput").ap()
    out = nc.dram_tensor("out", [BS, 2 * D], F32, kind="ExternalOutput").ap()

    with tile.TileContext(nc) as tc:
        _build(nc, tc, past, cand, maskt, tstamp, Wq, bq, Wk, bk, Wv, bv, Wi, bi, out)
    nc.compile()
    return nc


def _build(nc, tc, past, cand, maskt, tstamp, Wq, bq, Wk, bk, Wv, bv, Wi, bi, out):
    from contextlib import ExitStack

    ctx = ExitStack()
    consts = ctx.enter_context(tc.tile_pool(name="consts", bufs=1))
    xpool = ctx.enter_context(tc.tile_pool(name="x", bufs=1))
    prodp = ctx.enter_context(tc.tile_pool(name="prod", bufs=2))
    sm = ctx.enter_context(tc.tile_pool(name="sm", bufs=1))
    pp = ctx.enter_context(tc.tile_pool(name="pp", bufs=3, space="PSUM"))
    pxw = ctx.enter_context(tc.tile_pool(name="pxw", bufs=1, space="PSUM"))

    mm = nc.tensor.matmul

    # ---------------- constants / small loads ----------------
    # (emitted before the big x-stream DMAs so the scheduler orders the
    # small critical-path loads and the identity build first)
    ident = consts.tile([128, 128], F32)
    make_identity(nc, ident[:])

    Wq_sb = consts.tile([128, 2, D], F32)
    nc.scalar.dma_start(Wq_sb[:], Wq.rearrange("(c p) j -> p c j", p=128))
    Wk_sb = consts.tile([128, 2, D], F32)
    nc.scalar.dma_start(Wk_sb[:], Wk.rearrange("(c p) j -> p c j", p=128))
    cand_sb = consts.tile([BS, D], F32)
    nc.scalar.dma_start(cand_sb[:], cand)
    bq_row = consts.tile([1, D], F32)
    nc.scalar.dma_start(bq_row[:], bq.unsqueeze(0))
    bk_col = consts.tile([128, 2], F32)
    nc.scalar.dma_start(bk_col[:], bk.rearrange("(c p) -> p c", p=128))
    # mask / timestamps in (h,b) layout: [128, 100]
    mask_hb = consts.tile([128, HS], I32)
    nc.scalar.dma_start(mask_hb[:], _ap(maskt, 0, [[HS, 2], [S, BS], [1, HS]]))
    ts_hb = consts.tile([128, HS], F32)
    nc.scalar.dma_start(ts_hb[:], _ap(tstamp, 0, [[HS, 2], [S, BS], [1, HS]]))

    # m0 = 1 - mask; decay = exp(-td * t); WD constant halves. Emitted early
    # so the scheduler clears them off ACT/DVE before the per-chunk chain.
    m0_hb = consts.tile([128, HS], I32)
    nc.vector.tensor_scalar(m0_hb[:], mask_hb[:], -1.0, 1.0, ALU.mult, ALU.add)
    decay_hb = consts.tile([128, HS], F32)
    nc.scalar.activation(decay_hb[:], ts_hb[:], AF.Exp, scale=-TIME_DECAY)

    # pairmask[p, b] = 1 if p == b or p == 64 + b (partition-pair selector)
    pairmask = consts.tile([128, BS], BF16)
    nc.vector.tensor_add(pairmask[:], ident[:, 0:BS], ident[:, BS:128])
    # WD[p, s, 0:64] = pairmask * w_e[:, s];  WD[p, s, 64:128] = pairmask
    wds = []
    for i in range(2):
        wd = consts.tile([128, SC, 128], BF16, tag=f"wd{i}", name=f"wd{i}")
        wds.append(wd)
    for wd in wds:
        wd_pp = wd[:].ap[0][0]
        nc.scalar.copy(_ap(wd[:], 64, [[wd_pp, 128], [128, SC], [1, 64]]),
                       _ap(pairmask[:], 0, [[pairmask[:].ap[0][0], 128], [0, SC], [1, 64]]))

    # ---------------- x stream: SWDGE cast DMAs, (h,b) layout ----------------
    # partition p = 64h + b  <-  x[b, 100h + s, d]; per partition one
    # contiguous f32 run per chunk, cast to bf16 inline.
    xas = []
    for c in range(NCH):
        xa = xpool.tile([128, SC, D], BF16, tag=f"xa{c}")
        src = _ap(past, c * SC * D,
                  [[HS * D, 2], [S * D, BS], [D, SC], [1, D]])
        nc.gpsimd.dma_start(xa[:], src)
        xas.append(xa)

    ones_row = consts.tile([1, BS], F32)
    nc.vector.memset(ones_row[:], 1.0)
    neg_col = consts.tile([128, 1], F32)
    nc.vector.memset(neg_col[:], NEG_INF)

    # ---------------- phase A: q / r~ / c / candv ----------------
    candT = consts.tile([128, 2, BS], F32)
    for dc in range(2):
        pt = pp.tile([128, BS], F32, tag="pt")
        nc.tensor.transpose(pt[:], cand_sb[:, dc * 128:(dc + 1) * 128], ident[0:BS, 0:BS])
        nc.scalar.copy(candT[:, dc, :], pt[:])

    # WkT[u', uc, dc*128+d'] = Wk[dc*128+d', uc*128+u']
    WkT = consts.tile([128, 2, 2, 128], F32)
    for dc in range(2):
        for uc in range(2):
            pt = pp.tile([128, 128], F32, tag="pt")
            nc.tensor.transpose(pt[:], Wk_sb[:, dc, uc * 128:(uc + 1) * 128], ident[:])
            nc.scalar.copy(WkT[:, uc, dc, :], pt[:])

    # qT [u' (2 chunks), b] = Wq.T @ candT + bq
    qT = consts.tile([128, 2, BS], F32)
    for jc in range(2):
        pq = pp.tile([128, BS], F32, tag="pt")
        mm(pq[:], Wq_sb[:, 0, jc * 128:(jc + 1) * 128], candT[:, 0, :], start=True, stop=False)
        mm(pq[:], Wq_sb[:, 1, jc * 128:(jc + 1) * 128], candT[:, 1, :], start=False, stop=False)
        mm(pq[:], bq_row[0:1, jc * 128:(jc + 1) * 128], ones_row[:], start=False, stop=True)
        nc.scalar.copy(qT[:, jc, :], pq[:])

    # duplicated qT stationaries: qtd[:, uc, 64h+b] = q[b, uc*128+u']
    qtd = consts.tile([128, 2, 128], F32)
    for uc in range(2):
        nc.scalar.copy(qtd[:, uc, 0:64], qT[:, uc, :])
        nc.scalar.copy(qtd[:, uc, 64:128], qT[:, uc, :])

    # r~ [p=(h,b), d] = SCALE * sum_u q[b,u] Wk[d,u]   (bf16 for the DVE mult)
    rt_sb = consts.tile([128, D], BF16)
    pr = pp.tile([128, D], F32, tag="pt")
    mm(pr[:], qtd[:, 0, :], WkT[:, 0, :, :], start=True, stop=False)
    mm(pr[:], qtd[:, 1, :], WkT[:, 1, :, :], start=False, stop=True)
    nc.scalar.mul(rt_sb[:], pr[:], SCALE)

    # c_dup [p=(h,b), 1] = SCALE * (q . bk)
    c_dup = sm.tile([128, 1], F32)
    pc = pp.tile([128, 1], F32, tag="pt")
    mm(pc[:], qtd[:, 0, :], bk_col[:, 0:1], start=True, stop=False)
    mm(pc[:], qtd[:, 1, :], bk_col[:, 1:2], start=False, stop=True)
    nc.scalar.mul(c_dup[:], pc[:], SCALE)

    # finals-only loads
    Wv_sb = consts.tile([128, 2, D], F32)
    nc.scalar.dma_start(Wv_sb[:], Wv.rearrange("(c p) j -> p c j", p=128))
    Wi_sb = consts.tile([128, 8, D], F32)
    nc.scalar.dma_start(Wi_sb[:], Wi.rearrange("(c p) j -> p c j", p=128))
    bk_row = consts.tile([1, D], F32)
    nc.scalar.dma_start(bk_row[:], bk.unsqueeze(0))
    bv_row = consts.tile([1, D], F32)
    nc.scalar.dma_start(bv_row[:], bv.unsqueeze(0))
    bi_row = consts.tile([1, D], F32)
    nc.scalar.dma_start(bi_row[:], bi.unsqueeze(0))

    # candv = cand + bv (built on PE so no partition-broadcast is needed)
    pcv = pp.tile([BS, D], F32, tag="pt")
    for dc in range(2):
        dsl = slice(dc * 128, (dc + 1) * 128)
        mm(pcv[:, dsl], candT[:, dc, :], ident[:], start=True, stop=False)
        mm(pcv[:, dsl], ones_row[:], bv_row[0:1, dsl], start=False, stop=True)
    candv = consts.tile([BS, D], F32)
    nc.scalar.copy(candv[:], pcv[:])

    # ---------------- per-chunk: scores -> exp -> WD -> mms ----------
    s_raw = sm.tile([128, HS], FP16)
    s_f = sm.tile([128, HS], F32)
    w_e = sm.tile([128, HS], BF16)
    sums = sm.tile([128, NCH], F32)
    # psum accumulators per dc: [d', 0:64] = xw~T, [d', 64:128] = xsT (raw sums)
    px0 = pxw.tile([128, 128], F32, tag="px0")
    px1 = pxw.tile([128, 128], F32, tag="px1")
    pxs = [px0, px1]

    np_ = neg_col[:].ap[0][0]
    pm_pp = pairmask[:].ap[0][0]
    we_pp = w_e[:].ap[0][0]
    for c in range(NCH):
        L = slice(SC * c, SC * (c + 1))
        xa = xas[c]
        prod = prodp.tile([128, SC, D], BF16, tag="prod")
        rt_b = _ap(rt_sb[:], 0, [[rt_sb[:].ap[0][0], 128], [0, SC], [1, D]])
        nc.vector.tensor_mul(prod[:], xa[:], rt_b)
        # d-reduction as a binary tree of TensorTensor adds: TensorTensor runs
        # at the 2x 16-bit DVE rate while TensorReduce is 1x-only, so the tree
        # (2x geometric series) beats one reduce pass. fp16 temps keep the
        # partial-sum rounding small; |partials| < ~40, well inside fp16.
        t = prod
        w_ = D
        while w_ > 8:
            w_ //= 2
            nt = prodp.tile([128, SC, w_], FP16, tag=f"tr{w_}", name=f"tr{w_}")
            nc.vector.tensor_add(nt[:], t[:, :, 0:w_], t[:, :, w_:2 * w_])
            t = nt
        with nc.allow_low_precision(reason="fp16 raw scores, f32 internal accum"):
            nc.vector.tensor_reduce(s_raw[:, L], t[:], axis=AX.X, op=ALU.add)
        nc.scalar.add(s_f[:, L], s_raw[:, L], c_dup[:, 0:1])
        negb = _ap(neg_col[:], 0, [[np_, 128], [0, SC]])
        nc.vector.copy_predicated(s_f[:, L], m0_hb[:, L], negb)
        nc.vector.tensor_mul(s_f[:, L], s_f[:, L], decay_hb[:, L])
        nc.scalar.activation(w_e[:, L], s_f[:, L], AF.Exp, scale=1.0,
                             accum_out=sums[:, c:c + 1])
        # WD w-half: [128, SC, 64] = pairmask (bcast s) * w_e[:, L] (bcast col)
        wd = wds[c % 2]
        nc.gpsimd.tensor_tensor(
            wd[:, :, 0:64],
            _ap(pairmask[:], 0, [[pm_pp, 128], [0, SC], [1, 64]]),
            _ap(w_e[:], SC * c, [[we_pp, 128], [1, SC], [0, 64]]),
            op=ALU.mult)
        for sl in range(SC):
            s_ = SC * c + sl
            for dc in range(2):
                mm(pxs[dc][:], xa[:, sl, dc * 128:(dc + 1) * 128], wd[:, sl, :],
                   start=(s_ == 0), stop=(s_ == HS - 1))

    # ---------------- finals ----------------
    # softmax denominators: den[b] = sum over chunks and both halves
    sums_t = sm.tile([128, 1], F32)
    nc.vector.tensor_reduce(sums_t[:], sums[:], axis=AX.X, op=ALU.add)
    den = sm.tile([BS, 1], F32)
    nc.scalar.add(den[:], sums_t[0:BS, :], sums_t[BS:128, 0:1])
    rs = sm.tile([BS, 1], F32)
    nc.vector.reciprocal(rs[:], den[:])

    # extract psum: xwT (unnormalized) and xsT (scaled to mean)
    xwT = consts.tile([128, 2, BS], F32)
    xsT = consts.tile([128, 2, BS], F32)
    for dc in range(2):
        nc.scalar.copy(xwT[:, dc, :], pxs[dc][:, 0:BS])
        nc.scalar.mul(xsT[:, dc, :], pxs[dc][:, BS:128], 1.0 / S)

    # kbarT [d (2 chunks), b] = Wk.T @ xsT + bk
    kT = consts.tile([128, 2, BS], F32)
    for dc in range(2):
        pk = pp.tile([128, BS], F32, tag="pt")
        mm(pk[:], Wk_sb[:, 0, dc * 128:(dc + 1) * 128], xsT[:, 0, :], start=True, stop=False)
        mm(pk[:], Wk_sb[:, 1, dc * 128:(dc + 1) * 128], xsT[:, 1, :], start=False, stop=False)
        mm(pk[:], bk_row[0:1, dc * 128:(dc + 1) * 128], ones_row[:], start=False, stop=True)
        nc.scalar.copy(kT[:, dc, :], pk[:])

    fd = consts.tile([128, 2, BS], F32)
    fm = consts.tile([128, 2, BS], F32)
    for dc in range(2):
        nc.vector.tensor_sub(fd[:, dc, :], qT[:, dc, :], kT[:, dc, :])
        nc.vector.tensor_mul(fm[:, dc, :], qT[:, dc, :], kT[:, dc, :])

    # attended = (xw~ @ Wv) * rs + (cand + bv)
    pa = pp.tile([BS, D], F32, tag="pt")
    mm(pa[:], xwT[:, 0, :], Wv_sb[:, 0, :], start=True, stop=False)
    mm(pa[:], xwT[:, 1, :], Wv_sb[:, 1, :], start=False, stop=True)
    att_sb = sm.tile([BS, D], F32)
    nc.vector.tensor_scalar_mul(att_sb[:], pa[:], rs[:, 0:1])
    nc.vector.tensor_add(att_sb[:], att_sb[:], candv[:])
    nc.sync.dma_start(out[:, 0:D], att_sb[:])

    # inter_mean = feat @ Wi + bi
    pm = pp.tile([BS, D], F32, tag="pt")
    feat_chunks = [qT[:, 0, :], qT[:, 1, :], kT[:, 0, :], kT[:, 1, :],
                   fd[:, 0, :], fd[:, 1, :], fm[:, 0, :], fm[:, 1, :]]
    for ci, fc in enumerate(feat_chunks):
        mm(pm[:], fc, Wi_sb[:, ci, :], start=(ci == 0), stop=False)
    mm(pm[:], ones_row[:], bi_row[:], start=False, stop=True)
    mi_sb = sm.tile([BS, D], F32)
    nc.scalar.copy(mi_sb[:], pm[:])
    nc.sync.dma_start(out[:, D:2 * D], mi_sb[:])

    ctx.close()


_NC_CACHE = None


def _get_program():
    global _NC_CACHE
    if _NC_CACHE is None:
        _NC_CACHE = build_program()
    return _NC_CACHE


def _shard_inputs(inputs):
    in_maps = []
    for i in range(N_CORES):
        lo, hi = i * BS, (i + 1) * BS
        in_maps.append({
            "past_interactions": np.ascontiguousarray(inputs["past_interactions"][lo:hi]).astype(np.float32, copy=False),
            "candidate_embedding": np.ascontiguousarray(inputs["candidate_embedding"][lo:hi]).astype(np.float32, copy=False),
            "past_mask": np.ascontiguousarray(inputs["past_mask"][lo:hi]).astype(np.int32, copy=False),
            "past_timestamps": np.ascontiguousarray(inputs["past_timestamps"][lo:hi]).astype(np.float32, copy=False),
            "Wq": np.asarray(inputs["Wq"], np.float32),
            "bq": np.asarray(inputs["bq"], np.float32),
            "Wk": np.asarray(inputs["Wk"], np.float32),
            "bk": np.asarray(inputs["bk"], np.float32),
            "Wv": np.asarray(inputs["Wv"], np.float32),
            "bv": np.asarray(inputs["bv"], np.float32),
            "Wi": np.asarray(inputs["Wi"], np.float32),
            "bi": np.asarray(inputs["bi"], np.float32),
        })
    return in_maps


def run(inputs, trace=False):
    from concourse.bass_utils import run_bass_kernel_spmd

    nc = _get_program()
    in_maps = _shard_inputs(inputs)
    res = run_bass_kernel_spmd(nc, in_maps, list(range(N_CORES)), trace=trace)
    outs = [res.results[i]["out"] for i in range(N_CORES)]
    full = np.concatenate(outs, axis=0).astype(np.float32)
    return full, res


def kernel(**inputs):
    inputs = {k: np.asarray(v) for k, v in inputs.items()}
    full, _ = run(inputs, trace=False)
    return full


# revision 15
# speedup vs baseline: 12.7868x; 12.7868x over previous
"""Trainium2 Bass kernel for nn_EnhancedMultiHeadDINAttention.

Math (algebraically collapsed from the reference):
  q = cand @ Wq + bq                          [B, D]
  scores[b,s] = x[b,s,:] . r_b + c_b          r = (Wk @ q)*SCALE, c = (q.bk)*SCALE
  scores -> mask(-1e9) -> * decay -> exp -> w_e (unnormalized; the softmax
  denominator folds into a per-batch scale at the end)
  xw[b,:]  = sum_s w[b,s] x[b,s,:]
  xs[b,:]  = mean_s x[b,s,:]
  attended = xw @ Wv + bv + cand
  kbar     = xs @ Wk + bk
  inter_mean = concat([q, kbar, q-kbar, q*kbar]) @ Wi + bi
  out = concat([attended, inter_mean], -1)    [B, 2D]

Sharding: pure data parallel over batch, 8 cores x 64 batches.

Per-core layout: x streams HBM->SBUF once as bf16 (SWDGE cast in the DMA)
into [128, 100, 256] tiles where partition p = 64h + b holds batch b,
s-half h -- each partition reads one contiguous f32 run (max DMA
efficiency, no staging, no transpose of x anywhere).

scores: DVE broadcast-mult (x * r~, r~ replicated per partition pair) +
innermost reduce, both at the 2x bf16/fp16 rate.

xw/xs: PE matmuls with x d-slices as the *stationary* and a per-s masked
moving operand WD[:, s, 0:64] = pairmask * w_e[:, s] (diag pairs),
WD[:, s, 64:128] = pairmask. psum accumulates [d', (xw~T | xsT)] over all
s -- transposed exactly as the final Wv/Wk matmuls want, and the softmax
denominator is applied per-partition when extracting attended.
"""

import numpy as np

import concourse.bass as bass
import concourse.bacc as bacc
import concourse.tile as tile
from concourse import mybir
from concourse.masks import make_identity

F32 = mybir.dt.float32
BF16 = mybir.dt.bfloat16
FP16 = mybir.dt.float16
I32 = mybir.dt.int32

N_CORES = 8
B, S, D = 512, 200, 256
BS = B // N_CORES          # 64 batches per core
HS = S // 2                # 100 s-values per half
CHUNKS = [20, 20, 20, 20, 12, 8]   # x-load s-chunk sizes (sum = HS); the
NCH = len(CHUNKS)                  # tail chunks shrink so the post-DMA
SC = max(CHUNKS)                   # critical path is short
SCALE = 1.0 / np.sqrt(D // 4)   # 1/8
TIME_DECAY = 0.01
NEG_INF = -1e9

AF = mybir.ActivationFunctionType
ALU = mybir.AluOpType
AX = mybir.AxisListType


def _ap(ap, off, dims):
    return bass.AP(tensor=ap.tensor, offset=ap.offset + off, ap=dims)


def build_program():
    nc = bacc.Bacc("TRN2", target_bir_lowering=False, debug=False)

    past = nc.dram_tensor("past_interactions", [BS, S, D], F32, kind="ExternalInput").ap()
    cand = nc.dram_tensor("candidate_embedding", [BS, D], F32, kind="ExternalInput").ap()
    maskt = nc.dram_tensor("past_mask", [BS, S], I32, kind="ExternalInput").ap()
    tstamp = nc.dram_tensor("past_timestamps", [BS, S], F32, kind="ExternalInput").ap()
    Wq = nc.dram_tensor("Wq", [D, D], F32, kind="ExternalInput").ap()
    bq = nc.dram_tensor("bq", [D], F32, kind="ExternalInput").ap()
    Wk = nc.dram_tensor("Wk", [D, D], F32, kind="ExternalInput").ap()
    bk = nc.dram_tensor("bk", [D], F32, kind="ExternalInput").ap()
    Wv = nc.dram_tensor("Wv", [D, D], F32, kind="ExternalInput").ap()
    bv = nc.dram_tensor("bv", [D], F32, kind="ExternalInput").ap()
    Wi = nc.dram_tensor("Wi", [4 * D, D], F32, kind="ExternalInput").ap()
    bi = nc.dram_tensor("bi", [D], F32, kind="ExternalInput").ap()
    out = nc.dram_tensor("out", [BS, 2 * D], F32, kind="ExternalOutput").ap()

    with tile.TileContext(nc) as tc:
        _build(nc, tc, past, cand, maskt, tstamp, Wq, bq, Wk, bk, Wv, bv, Wi, bi, out)
    nc.compile()
    return nc


def _build(nc, tc, past, cand, maskt, tstamp, Wq, bq, Wk, bk, Wv, bv, Wi, bi, out):
    from contextlib import ExitStack

    ctx = ExitStack()
    consts = ctx.enter_context(tc.tile_pool(name="consts", bufs=1))
    xpool = ctx.enter_context(tc.tile_pool(name="x", bufs=1))
    prodp = ctx.enter_context(tc.tile_pool(name="prod", bufs=2))
    sm = ctx.enter_context(tc.tile_pool(name="sm", bufs=1))
    pp = ctx.enter_context(tc.tile_pool(name="pp", bufs=3, space="PSUM"))
    pxw = ctx.enter_context(tc.tile_pool(name="pxw", bufs=1, space="PSUM"))

    mm = nc.tensor.matmul

    # ---------------- constants / small loads ----------------
    # (emitted before the big x-stream DMAs so the scheduler orders the
    # small critical-path loads and the identity build first)
    ident = consts.tile([128, 128], F32)
    make_identity(nc, ident[:])

    Wq_sb = consts.tile([128, 2, D], F32)
    nc.scalar.dma_start(Wq_sb[:], Wq.rearrange("(c p) j -> p c j", p=128))
    Wk_sb = consts.tile([128, 2, D], F32)
    nc.scalar.dma_start(Wk_sb[:], Wk.rearrange("(c p) j -> p c j", p=128))
    cand_sb = consts.tile([BS, D], F32)
    nc.scalar.dma_start(cand_sb[:], cand)
    bq_row = consts.tile([1, D], F32)
    nc.scalar.dma_start(bq_row[:], bq.unsqueeze(0))
    bk_col = consts.tile([128, 2], F32)
    nc.scalar.dma_start(bk_col[:], bk.rearrange("(c p) -> p c", p=128))
    # mask / timestamps in (h,b) layout: [128, 100]
    mask_hb = consts.tile([128, HS], I32)
    nc.scalar.dma_start(mask_hb[:], _ap(maskt, 0, [[HS, 2], [S, BS], [1, HS]]))
    ts_hb = consts.tile([128, HS], F32)
    nc.scalar.dma_start(ts_hb[:], _ap(tstamp, 0, [[HS, 2], [S, BS], [1, HS]]))

    # m0 = 1 - mask; decay = exp(-td * t); WD constant halves. Emitted early
    # so the scheduler clears them off ACT/DVE before the per-chunk chain.
    m0_hb = consts.tile([128, HS], I32)
    nc.vector.tensor_scalar(m0_hb[:], mask_hb[:], -1.0, 1.0, ALU.mult, ALU.add)
    decay_hb = consts.tile([128, HS], F32)
    nc.scalar.activation(decay_hb[:], ts_hb[:], AF.Exp, scale=-TIME_DECAY)

    # pairmask[p, b] = 1 if p == b or p == 64 + b (partition-pair selector)
    pairmask = consts.tile([128, BS], BF16)
    nc.vector.tensor_add(pairmask[:], ident[:, 0:BS], ident[:, BS:128])
    # WD[p, s, 0:64] = pairmask * w_e[:, s];  WD[p, s, 64:128] = pairmask
    wds = []
    for i in range(2):
        wd = consts.tile([128, SC, 128], BF16, tag=f"wd{i}", name=f"wd{i}")
        wds.append(wd)
    for wd in wds:
        wd_pp = wd[:].ap[0][0]
        nc.scalar.copy(_ap(wd[:], 64, [[wd_pp, 128], [128, SC], [1, 64]]),
                       _ap(pairmask[:], 0, [[pairmask[:].ap[0][0], 128], [0, SC], [1, 64]]))
    nwd = len(wds)

    # ---------------- x stream: SWDGE cast DMAs, (h,b) layout ----------------
    # partition p = 64h + b  <-  x[b, 100h + s, d]; per partition one
    # contiguous f32 run per chunk, cast to bf16 inline.
    xas = []
    off = 0
    for c, sc in enumerate(CHUNKS):
        xa = xpool.tile([128, sc, D], BF16, tag=f"xa{c}", name=f"xa{c}")
        src = _ap(past, off * D,
                  [[HS * D, 2], [S * D, BS], [D, sc], [1, D]])
        nc.gpsimd.dma_start(xa[:], src)
        xas.append(xa)
        off += sc

    ones_row = consts.tile([1, BS], F32)
    nc.vector.memset(ones_row[:], 1.0)
    neg_col = consts.tile([128, 1], F32)
    nc.vector.memset(neg_col[:], NEG_INF)

    # ---------------- phase A: q / r~ / c / candv ----------------
    candT = consts.tile([128, 2, BS], F32)
    for dc in range(2):
        pt = pp.tile([128, BS], F32, tag="pt")
        nc.tensor.transpose(pt[:], cand_sb[:, dc * 128:(dc + 1) * 128], ident[0:BS, 0:BS])
        nc.scalar.copy(candT[:, dc, :], pt[:])

    # WkT[u', uc, dc*128+d'] = Wk[dc*128+d', uc*128+u']
    WkT = consts.tile([128, 2, 2, 128], F32)
    for dc in range(2):
        for uc in range(2):
            pt = pp.tile([128, 128], F32, tag="pt")
            nc.tensor.transpose(pt[:], Wk_sb[:, dc, uc * 128:(uc + 1) * 128], ident[:])
            nc.scalar.copy(WkT[:, uc, dc, :], pt[:])

    # qT [u' (2 chunks), b] = Wq.T @ candT + bq
    qT = consts.tile([128, 2, BS], F32)
    for jc in range(2):
        pq = pp.tile([128, BS], F32, tag="pt")
        mm(pq[:], Wq_sb[:, 0, jc * 128:(jc + 1) * 128], candT[:, 0, :], start=True, stop=False)
        mm(pq[:], Wq_sb[:, 1, jc * 128:(jc + 1) * 128], candT[:, 1, :], start=False, stop=False)
        mm(pq[:], bq_row[0:1, jc * 128:(jc + 1) * 128], ones_row[:], start=False, stop=True)
        nc.scalar.copy(qT[:, jc, :], pq[:])

    qTb = consts.tile([128, 2, BS], BF16)
    nc.scalar.copy(qTb[:], qT[:])

    # duplicated qT stationaries: qtd[:, uc, 64h+b] = q[b, uc*128+u']
    qtd = consts.tile([128, 2, 128], F32)
    for uc in range(2):
        nc.scalar.copy(qtd[:, uc, 0:64], qT[:, uc, :])
        nc.scalar.copy(qtd[:, uc, 64:128], qT[:, uc, :])

    # r~ [p=(h,b), d] = SCALE * sum_u q[b,u] Wk[d,u]   (bf16 for the DVE mult)
    rt_sb = consts.tile([128, D], BF16)
    pr = pp.tile([128, D], F32, tag="pt")
    mm(pr[:], qtd[:, 0, :], WkT[:, 0, :, :], start=True, stop=False)
    mm(pr[:], qtd[:, 1, :], WkT[:, 1, :, :], start=False, stop=True)
    nc.scalar.mul(rt_sb[:], pr[:], SCALE)

    # c_dup [p=(h,b), 1] = SCALE * (q . bk)
    c_dup = sm.tile([128, 1], F32)
    pc = pp.tile([128, 1], F32, tag="pt")
    mm(pc[:], qtd[:, 0, :], bk_col[:, 0:1], start=True, stop=False)
    mm(pc[:], qtd[:, 1, :], bk_col[:, 1:2], start=False, stop=True)
    nc.scalar.mul(c_dup[:], pc[:], SCALE)

    # finals-only loads; Wv/Wi go straight to bf16 (SWDGE cast) so the
    # final matmuls run at the 1-cycle/row bf16 rate instead of fp32's 4.
    Wv_sb = consts.tile([128, 2, D], BF16)
    nc.gpsimd.dma_start(Wv_sb[:], Wv.rearrange("(c p) j -> p c j", p=128))
    Wi_sb = consts.tile([128, 8, D], BF16)
    nc.gpsimd.dma_start(Wi_sb[:], Wi.rearrange("(c p) j -> p c j", p=128))
    Wk_sbb = consts.tile([128, 2, D], BF16)
    nc.scalar.copy(Wk_sbb[:], Wk_sb[:])
    bk_row = consts.tile([1, D], BF16)
    nc.gpsimd.dma_start(bk_row[:], bk.unsqueeze(0))
    bv_row = consts.tile([1, D], F32)
    nc.scalar.dma_start(bv_row[:], bv.unsqueeze(0))
    bi_row = consts.tile([1, D], BF16)
    nc.gpsimd.dma_start(bi_row[:], bi.unsqueeze(0))
    ones_rowb = consts.tile([1, BS], BF16)
    nc.vector.memset(ones_rowb[:], 1.0)

    # candv = cand + bv (built on PE so no partition-broadcast is needed)
    pcv = pp.tile([BS, D], F32, tag="pt")
    for dc in range(2):
        dsl = slice(dc * 128, (dc + 1) * 128)
        mm(pcv[:, dsl], candT[:, dc, :], ident[:], start=True, stop=False)
        mm(pcv[:, dsl], ones_row[:], bv_row[0:1, dsl], start=False, stop=True)
    candv = consts.tile([BS, D], F32)
    nc.scalar.copy(candv[:], pcv[:])

    # ---------------- per-chunk: scores -> exp -> WD -> mms ----------
    s_raw = sm.tile([128, HS], FP16)
    s_f = sm.tile([128, HS], F32)
    w_e = sm.tile([128, HS], BF16)
    sums = sm.tile([128, NCH], F32)
    # psum accumulators per dc: [d', 0:64] = xw~T, [d', 64:128] = xsT (raw sums)
    px0 = pxw.tile([128, 128], F32, tag="px0")
    px1 = pxw.tile([128, 128], F32, tag="px1")
    pxs = [px0, px1]

    np_ = neg_col[:].ap[0][0]
    pm_pp = pairmask[:].ap[0][0]
    we_pp = w_e[:].ap[0][0]
    off = 0
    for c, sc in enumerate(CHUNKS):
        L = slice(off, off + sc)
        xa = xas[c]
        prod = prodp.tile([128, sc, D], BF16, tag="prod", name="prod")
        rt_b = _ap(rt_sb[:], 0, [[rt_sb[:].ap[0][0], 128], [0, sc], [1, D]])
        nc.vector.tensor_mul(prod[:], xa[:], rt_b)
        # d-reduction as a binary tree of TensorTensor adds: TensorTensor runs
        # at the 2x 16-bit DVE rate while TensorReduce is 1x-only, so the tree
        # (2x geometric series) beats one reduce pass. fp16 temps keep the
        # partial-sum rounding small; |partials| < ~40, well inside fp16.
        t = prod
        w_ = D
        while w_ > 8:
            w_ //= 2
            nt = prodp.tile([128, sc, w_], FP16, tag=f"tr{w_}", name=f"tr{w_}")
            nc.vector.tensor_add(nt[:], t[:, :, 0:w_], t[:, :, w_:2 * w_])
            t = nt
        with nc.allow_low_precision(reason="fp16 raw scores, f32 internal accum"):
            nc.vector.tensor_reduce(s_raw[:, L], t[:], axis=AX.X, op=ALU.add)
        nc.scalar.add(s_f[:, L], s_raw[:, L], c_dup[:, 0:1])
        negb = _ap(neg_col[:], 0, [[np_, 128], [0, sc]])
        nc.vector.copy_predicated(s_f[:, L], m0_hb[:, L], negb)
        nc.vector.tensor_mul(s_f[:, L], s_f[:, L], decay_hb[:, L])
        nc.scalar.activation(w_e[:, L], s_f[:, L], AF.Exp, scale=1.0,
                             accum_out=sums[:, c:c + 1])
        # WD w-half: [128, sc, 64] = pairmask (bcast s) * w_e[:, L] (bcast col)
        # Late chunks build WD on DVE: Pool's 2.6us op would sit on the tail.
        wd = wds[c % nwd]
        wd_eng = nc.vector if c >= 4 else nc.gpsimd
        wd_pp2 = wd[:].ap[0][0]
        wd_eng.tensor_tensor(
            _ap(wd[:], 0, [[wd_pp2, 128], [128, sc], [1, 64]]),
            _ap(pairmask[:], 0, [[pm_pp, 128], [0, sc], [1, 64]]),
            _ap(w_e[:], off, [[we_pp, 128], [1, sc], [0, 64]]),
            op=ALU.mult)
        for sl in range(sc):
            s_ = off + sl
            for dc in range(2):
                mm(pxs[dc][:], xa[:, sl, dc * 128:(dc + 1) * 128], wd[:, sl, :],
                   start=(s_ == 0), stop=(s_ == HS - 1))
        off += sc

    # ---------------- finals ----------------
    # softmax denominators: den[b] = sum over chunks and both halves
    sums_t = sm.tile([128, 1], F32)
    nc.vector.tensor_reduce(sums_t[:], sums[:], axis=AX.X, op=ALU.add)
    den = sm.tile([BS, 1], F32)
    nc.scalar.add(den[:], sums_t[0:BS, :], sums_t[BS:128, 0:1])
    rs = sm.tile([BS, 1], F32)
    nc.vector.reciprocal(rs[:], den[:])

    # extract psum: xwT (unnormalized) and xsT (scaled to mean), bf16 for
    # the final matmuls
    xwT = consts.tile([128, 2, BS], BF16)
    xsT = consts.tile([128, 2, BS], BF16)
    for dc in range(2):
        nc.scalar.copy(xwT[:, dc, :], pxs[dc][:, 0:BS])
        nc.scalar.mul(xsT[:, dc, :], pxs[dc][:, BS:128], 1.0 / S)

    # kbarT [d (2 chunks), b] = Wk.T @ xsT + bk
    kT = consts.tile([128, 2, BS], BF16)
    for dc in range(2):
        pk = pp.tile([128, BS], F32, tag="pt")
        mm(pk[:], Wk_sbb[:, 0, dc * 128:(dc + 1) * 128], xsT[:, 0, :], start=True, stop=False)
        mm(pk[:], Wk_sbb[:, 1, dc * 128:(dc + 1) * 128], xsT[:, 1, :], start=False, stop=False)
        mm(pk[:], bk_row[0:1, dc * 128:(dc + 1) * 128], ones_rowb[:], start=False, stop=True)
        nc.scalar.copy(kT[:, dc, :], pk[:])

    fd = consts.tile([128, 2, BS], BF16)
    fm = consts.tile([128, 2, BS], BF16)
    for dc in range(2):
        nc.vector.tensor_sub(fd[:, dc, :], qTb[:, dc, :], kT[:, dc, :])
        nc.vector.tensor_mul(fm[:, dc, :], qTb[:, dc, :], kT[:, dc, :])

    # attended = (xw~ @ Wv) * rs + (cand + bv)
    pa = pp.tile([BS, D], F32, tag="pt")
    mm(pa[:], xwT[:, 0, :], Wv_sb[:, 0, :], start=True, stop=False)
    mm(pa[:], xwT[:, 1, :], Wv_sb[:, 1, :], start=False, stop=True)
    att_sb = sm.tile([BS, D], F32)
    nc.vector.tensor_scalar_mul(att_sb[:], pa[:], rs[:, 0:1])
    nc.vector.tensor_add(att_sb[:], att_sb[:], candv[:])
    nc.sync.dma_start(out[:, 0:D], att_sb[:])

    # inter_mean = feat @ Wi + bi
    pm = pp.tile([BS, D], F32, tag="pt")
    feat_chunks = [qTb[:, 0, :], qTb[:, 1, :], kT[:, 0, :], kT[:, 1, :],
                   fd[:, 0, :], fd[:, 1, :], fm[:, 0, :], fm[:, 1, :]]
    for ci, fc in enumerate(feat_chunks):
        mm(pm[:], fc, Wi_sb[:, ci, :], start=(ci == 0), stop=False)
    mm(pm[:], ones_rowb[:], bi_row[:], start=False, stop=True)
    mi_sb = sm.tile([BS, D], F32)
    nc.scalar.copy(mi_sb[:], pm[:])
    nc.sync.dma_start(out[:, D:2 * D], mi_sb[:])

    ctx.close()


_NC_CACHE = None


def _get_program():
    global _NC_CACHE
    if _NC_CACHE is None:
        _NC_CACHE = build_program()
    return _NC_CACHE


def _shard_inputs(inputs):
    in_maps = []
    for i in range(N_CORES):
        lo, hi = i * BS, (i + 1) * BS
        in_maps.append({
            "past_interactions": np.ascontiguousarray(inputs["past_interactions"][lo:hi]).astype(np.float32, copy=False),
            "candidate_embedding": np.ascontiguousarray(inputs["candidate_embedding"][lo:hi]).astype(np.float32, copy=False),
            "past_mask": np.ascontiguousarray(inputs["past_mask"][lo:hi]).astype(np.int32, copy=False),
            "past_timestamps": np.ascontiguousarray(inputs["past_timestamps"][lo:hi]).astype(np.float32, copy=False),
            "Wq": np.asarray(inputs["Wq"], np.float32),
            "bq": np.asarray(inputs["bq"], np.float32),
            "Wk": np.asarray(inputs["Wk"], np.float32),
            "bk": np.asarray(inputs["bk"], np.float32),
            "Wv": np.asarray(inputs["Wv"], np.float32),
            "bv": np.asarray(inputs["bv"], np.float32),
            "Wi": np.asarray(inputs["Wi"], np.float32),
            "bi": np.asarray(inputs["bi"], np.float32),
        })
    return in_maps


def run(inputs, trace=False):
    from concourse.bass_utils import run_bass_kernel_spmd

    nc = _get_program()
    in_maps = _shard_inputs(inputs)
    res = run_bass_kernel_spmd(nc, in_maps, list(range(N_CORES)), trace=trace)
    outs = [res.results[i]["out"] for i in range(N_CORES)]
    full = np.concatenate(outs, axis=0).astype(np.float32)
    return full, res


def kernel(**inputs):
    inputs = {k: np.asarray(v) for k, v in inputs.items()}
    full, _ = run(inputs, trace=False)
    return full
